# revision 1
# baseline (speedup 1.0000x reference)
"""Distributed Trainium2 Bass kernel for multi-head attention.

Reference computation (B=4, S=2048, D=1024, H=16 heads, HD=64):
    q = heads(Q @ Wq + bq + Q_lev)
    k = heads(K @ Wk + bk + K_lev)
    v = heads(V @ Wv + bv + V_lev)
    out = softmax(q k^T / sqrt(HD)) v  -> merge heads -> @ Wo + bo

Sharding: 8 cores = 4 batches x 2 query-halves (1024 queries each).
Each core computes its [1024, 1024] output slice end-to-end with zero
collectives; the K/V projections are recomputed by both cores of a
batch pair (cheaper than any 2-rank collective on this chip).

Device-side layout strategy (everything feature-major / pre-transposed
on the host so no on-chip transposes are needed):
  qT   [D, Sq]  = Wq.T @ Q.T   (+ bq + Q_lev, folded on host into qlevT)
  kT   [D, S]   = Wk.T @ K.T
  v    [S, D]   = V @ Wv       (stationary = V.T, moving = Wv)
  scoresT[keys, q] = kT_h.T @ qT_h        (contract over HD=64)
  probsT = exp(scoresT / 8)               (no max subtraction: scores are
                                           N(0,~2) so exp stays < ~1e6)
  ctxT_aug[65, q] = v_aug_h.T @ probsT    (v_aug has a 65th ones column,
                                           so row 64 = softmax denominator)
  ctxT = ctxT_aug[:64] * (1/denominator)  (bcast via a K=1 PE matmul)
  out[q, D] = ctxT.T @ Wo (+ bo)

Matmuls run in bf16 (f32 PSUM accumulation); the two K=64 scores
matmuls of a head pair are packed into PE row halves (base partitions
0/64) so the 128x128 array stays full.
"""

import os
import sys

import numpy as np

for _p in ("/opt/trn_rl_repo", "/root/.axon_site/_ro/trn_rl_repo"):
    if os.path.isdir(_p) and _p not in sys.path:
        sys.path.insert(0, _p)

import ml_dtypes  # noqa: E402

B, S, D, H = 4, 2048, 1024, 16
HD = D // H  # 64
SQ = S // 2  # queries per core
N_CORES = 8
P = 128  # SBUF partitions
DC = D // P  # 8 chunks of the feature dim
KC = S // P  # 16 key chunks
NB = 512  # matmul moving free-dim (one PSUM bank of f32)

_BUILD_CACHE = {}


def _build_nc():
    from concourse import bacc, bass, mybir, tile

    f32 = mybir.dt.float32
    bf16 = mybir.dt.bfloat16
    Exp = mybir.ActivationFunctionType.Exp

    nc = bacc.Bacc("TRN2", target_bir_lowering=False, debug=False, num_devices=N_CORES)

    qt_d = nc.dram_tensor("qt", [D, SQ], bf16, kind="ExternalInput")
    qlev_d = nc.dram_tensor("qlev", [D, SQ], f32, kind="ExternalInput")
    kt_d = nc.dram_tensor("kt", [D, S], bf16, kind="ExternalInput")
    klev_d = nc.dram_tensor("klev", [D, S], f32, kind="ExternalInput")
    vt_d = nc.dram_tensor("vt", [D, S], bf16, kind="ExternalInput")
    vlev_d = nc.dram_tensor("vlev", [S, D], f32, kind="ExternalInput")
    wq_d = nc.dram_tensor("wq", [D, D], bf16, kind="ExternalInput")
    wk_d = nc.dram_tensor("wk", [D, D], bf16, kind="ExternalInput")
    wv_d = nc.dram_tensor("wv", [D, D], bf16, kind="ExternalInput")
    wo_d = nc.dram_tensor("wo", [D, D], bf16, kind="ExternalInput")
    bo_d = nc.dram_tensor("bo_rep", [P, D], f32, kind="ExternalInput")
    out_d = nc.dram_tensor("out", [SQ, D], f32, kind="ExternalOutput")

    with tile.TileContext(nc) as tc:
        with tc.tile_pool(name="persist", bufs=1) as persist:
            # Persistent intermediates (bf16).
            qT = [persist.tile([P, SQ], bf16, name=f"qT{i}", tag=f"qT{i}") for i in range(DC)]
            kT = [persist.tile([P, S], bf16, name=f"kT{i}", tag=f"kT{i}") for i in range(DC)]
            # v with a ones column appended per head: [S, H, HD+1]
            vaug = [
                persist.tile([P, H, HD + 1], bf16, name=f"vaug{i}", tag=f"vaug{i}")
                for i in range(KC)
            ]
            ctxT = [persist.tile([P, SQ], bf16, name=f"ctxT{i}", tag=f"ctxT{i}") for i in range(DC)]
            ones_col = persist.tile([1, HD], bf16, name="ones_col", tag="ones_col")
            nc.vector.memset(ones_col[:], 1.0)

            # ---------------- Phase 1: projections ----------------
            with (
                tc.tile_pool(name="w1", bufs=1) as wpool,
                tc.tile_pool(name="in1", bufs=1) as inpool,
                tc.tile_pool(name="vst1", bufs=24) as vstp,
                tc.tile_pool(name="lev1", bufs=4) as levp,
                tc.tile_pool(name="ps1", bufs=4, space="PSUM") as ps1,
            ):
                wq_sb = [wpool.tile([P, D], bf16, name=f"wq{i}", tag=f"wq{i}") for i in range(DC)]
                wk_sb = [wpool.tile([P, D], bf16, name=f"wk{i}", tag=f"wk{i}") for i in range(DC)]
                wv_sb = [wpool.tile([P, D], bf16, name=f"wv{i}", tag=f"wv{i}") for i in range(DC)]
                for i in range(DC):
                    nc.sync.dma_start(wq_sb[i][:], wq_d[i * P : (i + 1) * P, :])
                    nc.sync.dma_start(wk_sb[i][:], wk_d[i * P : (i + 1) * P, :])
                    nc.sync.dma_start(wv_sb[i][:], wv_d[i * P : (i + 1) * P, :])

                # qT = Wq.T @ Q.T  (+ qlev, which already includes bq)
                qin = []
                for kc in range(DC):
                    t = inpool.tile([P, SQ], bf16, name="qin", tag="qin", bufs=DC)
                    nc.sync.dma_start(t[:], qt_d[kc * P : (kc + 1) * P, :])
                    qin.append(t)
                for n in range(SQ // NB):
                    for m in range(DC):
                        ps = ps1.tile([P, NB], f32, name="psq", tag="ps_proj")
                        for kc in range(DC):
                            nc.tensor.matmul(
                                ps[:],
                                wq_sb[kc][:, m * P : (m + 1) * P],
                                qin[kc][:, n * NB : (n + 1) * NB],
                                start=(kc == 0),
                                stop=(kc == DC - 1),
                            )
                        lev = levp.tile([P, NB], f32, name="levq", tag="lev")
                        nc.sync.dma_start(
                            lev[:], qlev_d[m * P : (m + 1) * P, n * NB : (n + 1) * NB]
                        )
                        nc.vector.tensor_add(qT[m][:, n * NB : (n + 1) * NB], ps[:], lev[:])

                # kT = Wk.T @ K.T  (+ klev, which already includes bk)
                kin = []
                for kc in range(DC):
                    t = inpool.tile([P, S], bf16, name="kin", tag="kin", bufs=DC)
                    nc.sync.dma_start(t[:], kt_d[kc * P : (kc + 1) * P, :])
                    kin.append(t)
                for n in range(S // NB):
                    for m in range(DC):
                        ps = ps1.tile([P, NB], f32, name="psk", tag="ps_proj")
                        for kc in range(DC):
                            nc.tensor.matmul(
                                ps[:],
                                wk_sb[kc][:, m * P : (m + 1) * P],
                                kin[kc][:, n * NB : (n + 1) * NB],
                                start=(kc == 0),
                                stop=(kc == DC - 1),
                            )
                        lev = levp.tile([P, NB], f32, name="levk", tag="lev")
                        nc.sync.dma_start(
                            lev[:], klev_d[m * P : (m + 1) * P, n * NB : (n + 1) * NB]
                        )
                        nc.vector.tensor_add(kT[m][:, n * NB : (n + 1) * NB], ps[:], lev[:])

                # v = V @ Wv (+ vlev, which already includes bv), written into
                # the head-strided vaug layout with ones columns.
                for m in range(KC):
                    vs = []
                    for kc in range(DC):
                        t = vstp.tile([P, P], bf16, name="vst", tag="vst")
                        nc.sync.dma_start(t[:], vt_d[kc * P : (kc + 1) * P, m * P : (m + 1) * P])
                        vs.append(t)
                    for n in range(D // NB):
                        ps = ps1.tile([P, NB], f32, name="psv", tag="ps_proj")
                        for kc in range(DC):
                            nc.tensor.matmul(
                                ps[:],
                                vs[kc][:],
                                wv_sb[kc][:, n * NB : (n + 1) * NB],
                                start=(kc == 0),
                                stop=(kc == DC - 1),
                            )
                        lev = levp.tile([P, NB], f32, name="levv", tag="lev")
                        nc.sync.dma_start(
                            lev[:], vlev_d[m * P : (m + 1) * P, n * NB : (n + 1) * NB]
                        )
                        hpb = NB // HD  # 8 heads per 512-col block
                        nc.vector.tensor_add(
                            vaug[m][:, n * hpb : (n + 1) * hpb, 0:HD],
                            ps[:].rearrange("p (h d) -> p h d", h=hpb),
                            lev[:].rearrange("p (h d) -> p h d", h=hpb),
                        )
                    nc.vector.memset(vaug[m][:, :, HD : HD + 1], 1.0)

            # ---------------- Phase 2: attention ----------------
            with (
                tc.tile_pool(name="probs2", bufs=6) as prp,
                tc.tile_pool(name="norm2", bufs=4) as nrm,
                tc.tile_pool(name="ps2", bufs=3, space="PSUM") as ps2,
                tc.tile_pool(name="ctxps2", bufs=2, space="PSUM") as ctxps,
                tc.tile_pool(name="bcps2", bufs=2, space="PSUM") as bcps,
            ):
                for qb in range(SQ // NB):
                    qs = slice(qb * NB, (qb + 1) * NB)
                    for hp in range(H // 2):  # head pair: heads 2hp, 2hp+1
                        cps = [
                            ctxps.tile([HD + 1, NB], f32, name=f"cps{e}", tag="ctxps")
                            for e in range(2)
                        ]
                        for kc in range(KC):
                            prs = []
                            sps_t = []
                            for e in range(2):
                                rows = slice(e * HD, (e + 1) * HD)
                                sps = ps2.tile([P, NB], f32, name="sps", tag="sps")
                                # scoresT[keys, q]; the e=1 head sits in PE
                                # array rows 64-127 (tile_position auto-derived
                                # from base partition) and runs concurrently.
                                nc.tensor.matmul(
                                    sps[:],
                                    kT[hp][rows, kc * P : (kc + 1) * P],
                                    qT[hp][rows, qs],
                                    start=True,
                                    stop=True,
                                )
                                sps_t.append(sps)
                            for e in range(2):
                                pr = prp.tile([P, NB], bf16, name="pr", tag="pr")
                                nc.scalar.activation(
                                    pr[:], sps_t[e][:], Exp, scale=1.0 / np.sqrt(HD)
                                )
                                prs.append(pr)
                            for e in range(2):
                                nc.tensor.matmul(
                                    cps[e][:],
                                    vaug[kc][:, 2 * hp + e, :],
                                    prs[e][:],
                                    start=(kc == 0),
                                    stop=(kc == KC - 1),
                                )
                        # normalize: ctxT_h = cps[:64] / cps[64]
                        for e in range(2):
                            rows = slice(e * HD, (e + 1) * HD)
                            recip_f = nrm.tile([1, NB], f32, name="recf", tag="recf")
                            nc.vector.reciprocal(recip_f[:], cps[e][HD : HD + 1, :])
                            recip_b = nrm.tile([1, NB], bf16, name="recb", tag="recb")
                            nc.scalar.copy(recip_b[:], recip_f[:])
                            bc = bcps.tile([HD, NB], f32, name="bc", tag="bc")
                            nc.tensor.matmul(
                                bc[:], ones_col[:], recip_b[:], start=True, stop=True
                            )
                            ctmp = nrm.tile([HD, NB], bf16, name="ctmp", tag="ctmp")
                            nc.scalar.copy(ctmp[:], cps[e][0:HD, :])
                            nc.vector.tensor_mul(ctxT[hp][rows, qs], ctmp[:], bc[:])

            # ---------------- Phase 3: output projection ----------------
            with (
                tc.tile_pool(name="w3", bufs=1) as w3,
                tc.tile_pool(name="out3", bufs=3) as outp,
                tc.tile_pool(name="ps3", bufs=4, space="PSUM") as ps3,
            ):
                wo_sb = [w3.tile([P, D], bf16, name=f"wo{i}", tag=f"wo{i}") for i in range(DC)]
                for i in range(DC):
                    nc.sync.dma_start(wo_sb[i][:], wo_d[i * P : (i + 1) * P, :])
                bo_sb = w3.tile([P, D], f32, name="bo_sb", tag="bo_sb")
                nc.sync.dma_start(bo_sb[:], bo_d[:])

                for qc in range(SQ // P):
                    for n in range(D // NB):
                        ps = ps3.tile([P, NB], f32, name="pso", tag="pso")
                        for dc in range(DC):
                            nc.tensor.matmul(
                                ps[:],
                                ctxT[dc][:, qc * P : (qc + 1) * P],
                                wo_sb[dc][:, n * NB : (n + 1) * NB],
                                start=(dc == 0),
                                stop=(dc == DC - 1),
                            )
                        ot = outp.tile([P, NB], f32, name="ot", tag="ot")
                        nc.vector.tensor_add(ot[:], ps[:], bo_sb[:, n * NB : (n + 1) * NB])
                        nc.sync.dma_start(
                            out_d[qc * P : (qc + 1) * P, n * NB : (n + 1) * NB], ot[:]
                        )

    nc.compile()
    return nc


def get_nc():
    if "nc" not in _BUILD_CACHE:
        _BUILD_CACHE["nc"] = _build_nc()
    return _BUILD_CACHE["nc"]


def make_in_maps(inputs):
    bf16 = ml_dtypes.bfloat16
    f32 = np.float32
    Q = np.asarray(inputs["Q"], f32)
    Q_lev = np.asarray(inputs["Q_lev"], f32)
    K = np.asarray(inputs["K"], f32)
    K_lev = np.asarray(inputs["K_lev"], f32)
    V = np.asarray(inputs["V"], f32)
    V_lev = np.asarray(inputs["V_lev"], f32)
    bq = np.asarray(inputs["bq"], f32)
    bk = np.asarray(inputs["bk"], f32)
    bv = np.asarray(inputs["bv"], f32)
    bo = np.asarray(inputs["bo"], f32)

    shared = {
        "wq": np.ascontiguousarray(np.asarray(inputs["Wq"], f32).astype(bf16)),
        "wk": np.ascontiguousarray(np.asarray(inputs["Wk"], f32).astype(bf16)),
        "wv": np.ascontiguousarray(np.asarray(inputs["Wv"], f32).astype(bf16)),
        "wo": np.ascontiguousarray(np.asarray(inputs["Wo"], f32).astype(bf16)),
        "bo_rep": np.ascontiguousarray(np.tile(bo.reshape(1, -1), (P, 1)).astype(f32)),
    }
    per_batch = []
    for b in range(B):
        per_batch.append(
            {
                "kt": np.ascontiguousarray(K[b].T.astype(bf16)),
                "klev": np.ascontiguousarray((K_lev[b] + bk).T).astype(f32),
                "vt": np.ascontiguousarray(V[b].T.astype(bf16)),
                "vlev": np.ascontiguousarray(V_lev[b] + bv).astype(f32),
            }
        )
    in_maps = []
    for c in range(N_CORES):
        b, hf = divmod(c, 2)
        qs = slice(hf * SQ, (hf + 1) * SQ)
        in_maps.append(
            {
                "qt": np.ascontiguousarray(Q[b, qs, :].T.astype(bf16)),
                "qlev": np.ascontiguousarray((Q_lev[b, qs, :] + bq).T).astype(f32),
                **per_batch[b],
                **shared,
            }
        )
    return in_maps


def run_on_cores(inputs, trace=False):
    """Run the SPMD kernel; returns (full_output, BassKernelResults)."""
    from concourse.bass_utils import run_bass_kernel_spmd

    nc = get_nc()
    in_maps = make_in_maps(inputs)
    res = run_bass_kernel_spmd(nc, in_maps, core_ids=list(range(N_CORES)), trace=trace)
    out = np.empty((B, S, D), np.float32)
    for c in range(N_CORES):
        b, hf = divmod(c, 2)
        out[b, hf * SQ : (hf + 1) * SQ, :] = res.results[c]["out"]
    return out, res


def kernel(**inputs):
    out, _ = run_on_cores(inputs, trace=False)
    return out


if __name__ == "__main__":
    nc = get_nc()
    print("built + compiled OK")


# revision 6
# speedup vs baseline: 1.9767x; 1.9767x over previous
"""Distributed Trainium2 Bass kernel for multi-head attention.

Reference computation (B=4, S=2048, D=1024, H=16 heads, HD=64):
    q = heads(Q @ Wq + bq + Q_lev)
    k = heads(K @ Wk + bk + K_lev)
    v = heads(V @ Wv + bv + V_lev)
    out = softmax(q k^T / sqrt(HD)) v  -> merge heads -> @ Wo + bo

Sharding: 8 cores = 4 batches x 2 query-halves (1024 queries each).
Each core computes its [1024, 1024] output slice end-to-end with zero
collectives; the K/V projections are recomputed by both cores of a
batch pair (cheaper than any 2-rank collective on this chip).

Device-side layout strategy (everything feature-major / pre-transposed
on the host so no on-chip transposes are needed):
  qT   [D, Sq]  = Wq.T @ Q.T   (+ bq + Q_lev, folded on host into qlevT)
  kT   [D, S]   = Wk.T @ K.T
  v    [S, D]   = V @ Wv       (stationary = V.T, moving = Wv)
  scoresT[keys, q] = kT_h.T @ qT_h        (contract over HD=64)
  probsT = exp(scoresT / 8)               (no max subtraction: scores are
                                           N(0,~2) so exp stays < ~1e6)
  ctxT_aug[65, q] = v_aug_h.T @ probsT    (v_aug has a 65th ones column,
                                           so row 64 = softmax denominator)
  ctxT = ctxT_aug[:64] * (1/denominator)  (batched reciprocal + block-diag
                                           ones matmul to broadcast 1/sum
                                           across the 64 head-dim partitions)
  out[q, D] = ctxT.T @ Wo (+ bo)

Matmuls run in bf16 (f32 PSUM accumulation). The two K=64 scores
matmuls of a head pair run concurrently in PE row halves (tile_position
auto-derived from base partitions 0/64) and write the two banks of one
[128, 1024] PSUM tile so a single wide ACT exp serves both heads.
The PE instruction stream is software-pipelined: scores(kc+1) issues
before ctx(kc) so the PE never stalls on the exp of the current tile.
"""

import os
import sys

import numpy as np

for _p in ("/opt/trn_rl_repo", "/root/.axon_site/_ro/trn_rl_repo"):
    if os.path.isdir(_p) and _p not in sys.path:
        sys.path.insert(0, _p)

import ml_dtypes  # noqa: E402

B, S, D, H = 4, 2048, 1024, 16
HD = D // H  # 64
SQ = S // 2  # queries per core
N_CORES = 8
P = 128  # SBUF partitions
DC = D // P  # 8 chunks of the feature dim
KC = S // P  # 16 key chunks
NB = 512  # matmul moving free-dim (one PSUM bank of f32)

_BUILD_CACHE = {}


def _build_nc():
    from concourse import bacc, mybir, tile

    f32 = mybir.dt.float32
    bf16 = mybir.dt.bfloat16
    Exp = mybir.ActivationFunctionType.Exp

    nc = bacc.Bacc("TRN2", target_bir_lowering=False, debug=False, num_devices=N_CORES)

    qt_d = nc.dram_tensor("qt", [D, SQ], bf16, kind="ExternalInput")
    qlev_d = nc.dram_tensor("qlev", [D, SQ], f32, kind="ExternalInput")
    kt_d = nc.dram_tensor("kt", [D, S], bf16, kind="ExternalInput")
    klev_d = nc.dram_tensor("klev", [D, S], f32, kind="ExternalInput")
    vt_d = nc.dram_tensor("vt", [D, S], bf16, kind="ExternalInput")
    vlev_d = nc.dram_tensor("vlev", [S, D], f32, kind="ExternalInput")
    wq_d = nc.dram_tensor("wq", [D, D], bf16, kind="ExternalInput")
    wk_d = nc.dram_tensor("wk", [D, D], bf16, kind="ExternalInput")
    wv_d = nc.dram_tensor("wv", [D, D], bf16, kind="ExternalInput")
    wo_d = nc.dram_tensor("wo", [D, D], bf16, kind="ExternalInput")
    bo_d = nc.dram_tensor("bo_rep", [P, D], f32, kind="ExternalInput")
    ones16_d = nc.dram_tensor("ones16", [H, D], bf16, kind="ExternalInput")
    out_d = nc.dram_tensor("out", [SQ, D], f32, kind="ExternalOutput")

    with tile.TileContext(nc) as tc:
        with (
            tc.tile_pool(name="persist", bufs=1) as persist,
            tc.tile_pool(name="wpool", bufs=16) as wpool,
            tc.tile_pool(name="w3", bufs=1) as w3p,
            tc.tile_pool(name="inp", bufs=18) as inp,
            tc.tile_pool(name="vst", bufs=16) as vstp,
            tc.tile_pool(name="lev", bufs=3) as levp,
            tc.tile_pool(name="probs", bufs=4) as prp,
            tc.tile_pool(name="norm", bufs=2) as nrm,
            tc.tile_pool(name="outp", bufs=2) as outp,
            tc.tile_pool(name="psum", bufs=1, space="PSUM") as psum,
        ):
            # Persistent intermediates (bf16).
            qT = [persist.tile([P, SQ], bf16, name=f"qT{i}", tag=f"qT{i}") for i in range(DC)]
            kT = [persist.tile([P, S], bf16, name=f"kT{i}", tag=f"kT{i}") for i in range(DC)]
            vaug = [
                persist.tile([P, H, HD + 1], bf16, name=f"vaug{i}", tag=f"vaug{i}")
                for i in range(KC)
            ]
            ctxT = [persist.tile([P, SQ], bf16, name=f"ctxT{i}", tag=f"ctxT{i}") for i in range(DC)]
            # Block-diagonal ones [16, D]: ones16[h, m] = 1 iff m // 64 == h.
            # Used to broadcast the per-(head, q) reciprocal across the 64
            # head-dim partitions via a K=16 matmul. Built on the host
            # (engine APs may only start at partitions 0/32/64/96, so it
            # cannot be memset per-head on chip).
            ones16 = persist.tile([H, D], bf16, name="ones16", tag="ones16")
            nc.sync.dma_start(ones16[:], ones16_d[:])

            # ---------------- Phase 1: projections ----------------
            wq_sb = [wpool.tile([P, D], bf16, name=f"wq{i}", tag="w") for i in range(DC)]
            wk_sb = [wpool.tile([P, D], bf16, name=f"wk{i}", tag="w") for i in range(DC)]
            for i in range(DC):
                nc.sync.dma_start(wq_sb[i][:], wq_d[i * P : (i + 1) * P, :])
            for i in range(DC):
                nc.sync.dma_start(wk_sb[i][:], wk_d[i * P : (i + 1) * P, :])

            # qT = Wq.T @ Q.T  (+ qlev, which already includes bq)
            with nc.named_scope("proj_q"):
                for n in range(SQ // NB):
                    qin = []
                    for kc in range(DC):
                        t = inp.tile([P, NB], bf16, name="qin", tag="qkin")
                        nc.sync.dma_start(
                            t[:], qt_d[kc * P : (kc + 1) * P, n * NB : (n + 1) * NB]
                        )
                        qin.append(t)
                    for m in range(DC):
                        ps = psum.tile([P, NB], f32, name="psq", tag="ps_proj", bufs=2)
                        for kc in range(DC):
                            nc.tensor.matmul(
                                ps[:],
                                wq_sb[kc][:, m * P : (m + 1) * P],
                                qin[kc][:],
                                start=(kc == 0),
                                stop=(kc == DC - 1),
                            )
                        lev = levp.tile([P, NB], f32, name="levq", tag="lev")
                        nc.sync.dma_start(
                            lev[:], qlev_d[m * P : (m + 1) * P, n * NB : (n + 1) * NB]
                        )
                        nc.vector.tensor_add(qT[m][:, n * NB : (n + 1) * NB], ps[:], lev[:])

            # kT = Wk.T @ K.T  (+ klev, which already includes bk)
            with nc.named_scope("proj_k"):
                for n in range(S // NB):
                    kin = []
                    for kc in range(DC):
                        t = inp.tile([P, NB], bf16, name="kin", tag="qkin")
                        nc.sync.dma_start(
                            t[:], kt_d[kc * P : (kc + 1) * P, n * NB : (n + 1) * NB]
                        )
                        kin.append(t)
                    for m in range(DC):
                        ps = psum.tile([P, NB], f32, name="psk", tag="ps_proj", bufs=2)
                        for kc in range(DC):
                            nc.tensor.matmul(
                                ps[:],
                                wk_sb[kc][:, m * P : (m + 1) * P],
                                kin[kc][:],
                                start=(kc == 0),
                                stop=(kc == DC - 1),
                            )
                        lev = levp.tile([P, NB], f32, name="levk", tag="lev")
                        nc.sync.dma_start(
                            lev[:], klev_d[m * P : (m + 1) * P, n * NB : (n + 1) * NB]
                        )
                        nc.vector.tensor_add(kT[m][:, n * NB : (n + 1) * NB], ps[:], lev[:])

            # v = V @ Wv (+ vlev, which already includes bv), written into the
            # head-strided vaug layout with ones columns.
            with nc.named_scope("proj_v"):
                wv_sb = [wpool.tile([P, D], bf16, name=f"wv{i}", tag="w") for i in range(DC)]
                for i in range(DC):
                    nc.sync.dma_start(wv_sb[i][:], wv_d[i * P : (i + 1) * P, :])
                for m in range(KC):
                    vs = []
                    for kc in range(DC):
                        t = vstp.tile([P, P], bf16, name="vst", tag="vst")
                        nc.sync.dma_start(
                            t[:], vt_d[kc * P : (kc + 1) * P, m * P : (m + 1) * P]
                        )
                        vs.append(t)
                    for n in range(D // NB):
                        ps = psum.tile([P, NB], f32, name="psv", tag="ps_proj", bufs=2)
                        for kc in range(DC):
                            nc.tensor.matmul(
                                ps[:],
                                vs[kc][:],
                                wv_sb[kc][:, n * NB : (n + 1) * NB],
                                start=(kc == 0),
                                stop=(kc == DC - 1),
                            )
                        lev = levp.tile([P, NB], f32, name="levv", tag="lev")
                        nc.sync.dma_start(
                            lev[:], vlev_d[m * P : (m + 1) * P, n * NB : (n + 1) * NB]
                        )
                        hpb = NB // HD  # 8 heads per 512-col block
                        nc.vector.tensor_add(
                            vaug[m][:, n * hpb : (n + 1) * hpb, 0:HD],
                            ps[:].rearrange("p (h d) -> p h d", h=hpb),
                            lev[:].rearrange("p (h d) -> p h d", h=hpb),
                        )
                    nc.vector.memset(vaug[m][:, :, HD : HD + 1], 1.0)

            # Phase 3 weights (loaded early; DMA is not the bottleneck).
            wo_sb = [w3p.tile([P, D], bf16, name=f"wo{i}", tag=f"wo{i}") for i in range(DC)]
            for i in range(DC):
                nc.sync.dma_start(wo_sb[i][:], wo_d[i * P : (i + 1) * P, :])
            bo_sb = w3p.tile([P, D], f32, name="bo_sb", tag="bo_sb")
            nc.sync.dma_start(bo_sb[:], bo_d[:])

            # -------- Phase 2 + 3: attention + output projection, per qb ----
            for qb in range(SQ // NB):
                qs = slice(qb * NB, (qb + 1) * NB)
                sums = nrm.tile([H, NB], f32, name="sums", tag="sums")
                with nc.named_scope(f"attn_qb{qb}"):
                    for hp in range(H // 2):  # head pair: heads 2hp, 2hp+1
                        cps = [
                            psum.tile([HD + 1, NB], f32, name=f"cps{e}", tag="ctxps", bufs=2)
                            for e in range(2)
                        ]
                        # software pipeline: scores(kc) ; exp(kc) ; ctx(kc-1)
                        sp_hist = [None, None]  # probs tiles for kc-1
                        for kc in range(KC + 1):
                            if kc < KC:
                                sps = psum.tile([P, 2 * NB], f32, name="sps", tag="sps", bufs=2)
                                for e in range(2):
                                    rows = slice(e * HD, (e + 1) * HD)
                                    # head pair packed in PE row halves
                                    nc.tensor.matmul(
                                        sps[:, e * NB : (e + 1) * NB],
                                        kT[hp][rows, kc * P : (kc + 1) * P],
                                        qT[hp][rows, qs],
                                        start=True,
                                        stop=True,
                                    )
                                pr = prp.tile([P, 2 * NB], bf16, name="pr", tag="pr")
                                nc.scalar.activation(pr[:], sps[:], Exp, scale=1.0 / 8.0)
                            if kc > 0:
                                pkc = kc - 1
                                ppr = sp_hist
                                for e in range(2):
                                    nc.tensor.matmul(
                                        cps[e][:],
                                        vaug[pkc][:, 2 * hp + e, :],
                                        ppr[0][:, e * NB : (e + 1) * NB],
                                        start=(pkc == 0),
                                        stop=(pkc == KC - 1),
                                    )
                            if kc < KC:
                                sp_hist = [pr, None]
                        for e in range(2):
                            h = 2 * hp + e
                            rows = slice(e * HD, (e + 1) * HD)
                            # Stash the denominator row: engines cannot write
                            # partition h directly (bases limited to
                            # 0/32/64/96), so stage on partition 64 in SBUF
                            # then DMA (partition-agnostic) into sums[h].
                            stg = nrm.tile([HD + 1, NB], f32, name="stg", tag="stg")
                            nc.vector.tensor_copy(stg[HD : HD + 1, :], cps[e][HD : HD + 1, :])
                            nc.sync.dma_start(sums[h : h + 1, :], stg[HD : HD + 1, :])
                            # copy unnormalized ctx (normalized in place later)
                            nc.vector.tensor_copy(ctxT[hp][rows, qs], cps[e][0:HD, :])
                    # batched normalization for all 16 heads of this q-block
                    recip_f = nrm.tile([H, NB], f32, name="recf", tag="recf")
                    nc.vector.reciprocal(recip_f[:], sums[:])
                    recip_b = nrm.tile([H, NB], bf16, name="recb", tag="recb")
                    nc.scalar.copy(recip_b[:], recip_f[:])
                    for dc in range(DC):
                        bc = psum.tile([P, NB], f32, name="bc", tag="ps_proj", bufs=2)
                        nc.tensor.matmul(
                            bc[:],
                            ones16[:, dc * P : (dc + 1) * P],
                            recip_b[:],
                            start=True,
                            stop=True,
                        )
                        nc.vector.tensor_mul(ctxT[dc][:, qs], ctxT[dc][:, qs], bc[:])

                # ---- output projection for this q-block ----
                with nc.named_scope(f"outproj_qb{qb}"):
                    for qc in range(NB // P):
                        qg = qb * (NB // P) + qc  # global 128-query chunk
                        for n in range(D // NB):
                            ps = psum.tile([P, NB], f32, name="pso", tag="ps_proj", bufs=2)
                            for dc in range(DC):
                                nc.tensor.matmul(
                                    ps[:],
                                    ctxT[dc][:, qg * P : (qg + 1) * P],
                                    wo_sb[dc][:, n * NB : (n + 1) * NB],
                                    start=(dc == 0),
                                    stop=(dc == DC - 1),
                                )
                            ot = outp.tile([P, NB], f32, name="ot", tag="ot")
                            nc.vector.tensor_add(ot[:], ps[:], bo_sb[:, n * NB : (n + 1) * NB])
                            nc.sync.dma_start(
                                out_d[qg * P : (qg + 1) * P, n * NB : (n + 1) * NB], ot[:]
                            )

    nc.compile()
    return nc


def get_nc():
    if "nc" not in _BUILD_CACHE:
        _BUILD_CACHE["nc"] = _build_nc()
    return _BUILD_CACHE["nc"]


def make_in_maps(inputs):
    bf16 = ml_dtypes.bfloat16
    f32 = np.float32
    Q = np.asarray(inputs["Q"], f32)
    Q_lev = np.asarray(inputs["Q_lev"], f32)
    K = np.asarray(inputs["K"], f32)
    K_lev = np.asarray(inputs["K_lev"], f32)
    V = np.asarray(inputs["V"], f32)
    V_lev = np.asarray(inputs["V_lev"], f32)
    bq = np.asarray(inputs["bq"], f32)
    bk = np.asarray(inputs["bk"], f32)
    bv = np.asarray(inputs["bv"], f32)
    bo = np.asarray(inputs["bo"], f32)

    shared = {
        "wq": np.ascontiguousarray(np.asarray(inputs["Wq"], f32).astype(bf16)),
        "wk": np.ascontiguousarray(np.asarray(inputs["Wk"], f32).astype(bf16)),
        "wv": np.ascontiguousarray(np.asarray(inputs["Wv"], f32).astype(bf16)),
        "wo": np.ascontiguousarray(np.asarray(inputs["Wo"], f32).astype(bf16)),
        "bo_rep": np.ascontiguousarray(np.tile(bo.reshape(1, -1), (P, 1)).astype(f32)),
        "ones16": np.kron(np.eye(H, dtype=f32), np.ones((1, HD), f32)).astype(bf16),
    }
    per_batch = []
    for b in range(B):
        per_batch.append(
            {
                "kt": np.ascontiguousarray(K[b].T.astype(bf16)),
                "klev": np.ascontiguousarray((K_lev[b] + bk).T).astype(f32),
                "vt": np.ascontiguousarray(V[b].T.astype(bf16)),
                "vlev": np.ascontiguousarray(V_lev[b] + bv).astype(f32),
            }
        )
    in_maps = []
    for c in range(N_CORES):
        b, hf = divmod(c, 2)
        qs = slice(hf * SQ, (hf + 1) * SQ)
        in_maps.append(
            {
                "qt": np.ascontiguousarray(Q[b, qs, :].T.astype(bf16)),
                "qlev": np.ascontiguousarray((Q_lev[b, qs, :] + bq).T).astype(f32),
                **per_batch[b],
                **shared,
            }
        )
    return in_maps


def run_on_cores(inputs, trace=False):
    """Run the SPMD kernel; returns (full_output, BassKernelResults)."""
    from concourse.bass_utils import run_bass_kernel_spmd

    nc = get_nc()
    in_maps = make_in_maps(inputs)
    res = run_bass_kernel_spmd(nc, in_maps, core_ids=list(range(N_CORES)), trace=trace)
    out = np.empty((B, S, D), np.float32)
    for c in range(N_CORES):
        b, hf = divmod(c, 2)
        out[b, hf * SQ : (hf + 1) * SQ, :] = res.results[c]["out"]
    return out, res


def kernel(**inputs):
    out, _ = run_on_cores(inputs, trace=False)
    return out


if __name__ == "__main__":
    nc = get_nc()
    print("built + compiled OK")
